# revision 12
# baseline (speedup 1.0000x reference)
"""Trainium2 Bass kernel for gated graph-attention (nn_Att_23502061043766).

Computation (reference, fp32):
    q = h @ Qw.T + Qb; k = h @ Kw.T + Kb; v = h @ Vw.T + Vb
    gate = sigmoid(e @ Gw.T + Gb); pe = e @ Pw.T + Pb
    scores = q @ k.T / sqrt(128) + pe
    attm = softmax(scores, axis=-1); attm = dropout(attm, p=0.2, key=42)
    out = (gate * attm) @ v

Strategy: shard rows of e/scores across the 8 cores (768 rows each). On
each core everything is computed in TRANSPOSED layout sT[j, i] (j = key
index on partitions, i = query rows of this core on the free axis), so:
  - the big [N,N]x[N,N] matmuls stream Pw.T/Gw.T tiles as the stationary
    operand and cached e.T tiles as the moving operand (each weight byte is
    read from HBM exactly once per core),
  - the Pb/Gb biases become per-partition biases folded into the ScalarE
    activation (Exp / Sigmoid) that evicts PSUM,
  - softmax needs no transposes: scores are ~N(0,2) so exp() without the
    row-max subtraction is safe in fp32; the row sum is a ones-vector
    matmul; attention-prob tiles are already [j, i] = exactly the layout
    the final (gate*attm) @ v matmul needs as its moving operand.
All big operands are bf16 (PSUM accumulates fp32).
"""

import sys

for _p in ("/opt/trn_rl_repo",):
    if _p not in sys.path:
        sys.path.append(_p)

import numpy as np
import ml_dtypes

BF = ml_dtypes.bfloat16
N = 6144
D = 128
NCORES = 8
R = N // NCORES          # 768 rows per core
CB = N // 128            # 48 contraction blocks
JB = N // 128            # 48 j blocks
I0, I1 = 512, 256        # i-chunk split of R (PSUM bank is 512 fp32)
DROPOUT_P = 0.2
DROP_KEY = 42

_cache = {}


def _build_nc():
    import concourse.bass as bass  # noqa: F401
    import concourse.tile as tile
    from concourse import bacc, mybir
    from contextlib import ExitStack

    f32 = mybir.dt.float32
    bf16 = mybir.dt.bfloat16
    AF = mybir.ActivationFunctionType
    MUL = mybir.AluOpType.mult

    nc = bacc.Bacc("TRN2", target_bir_lowering=False, debug=False,
                   num_devices=NCORES)

    # DRAM I/O (per-core shapes; per-core data differs only for eT/mk/hq)
    eT_d = nc.dram_tensor("eT", [N, R], bf16, kind="ExternalInput")
    pg_d = nc.dram_tensor("pg", [JB, N, 256], bf16, kind="ExternalInput")
    mk_d = nc.dram_tensor("mk", [N, R], bf16, kind="ExternalInput")
    hT_d = nc.dram_tensor("hT", [D, N], bf16, kind="ExternalInput")
    hq_d = nc.dram_tensor("hq", [D, R], bf16, kind="ExternalInput")
    kwT_d = nc.dram_tensor("kwT", [D, D], bf16, kind="ExternalInput")
    qwT_d = nc.dram_tensor("qwT", [D, D], bf16, kind="ExternalInput")
    vwT_d = nc.dram_tensor("vwT", [D, D], bf16, kind="ExternalInput")
    kb_d = nc.dram_tensor("kb", [D, 1], f32, kind="ExternalInput")
    qb_d = nc.dram_tensor("qb", [D, 1], f32, kind="ExternalInput")
    vb_d = nc.dram_tensor("vb", [1, D], bf16, kind="ExternalInput")
    pb_d = nc.dram_tensor("pb", [D, JB], f32, kind="ExternalInput")
    gb_d = nc.dram_tensor("gb", [D, JB], f32, kind="ExternalInput")
    outT_d = nc.dram_tensor("outT", [D, R], f32, kind="ExternalOutput")
    rsum_d = nc.dram_tensor("rsum", [1, R], f32, kind="ExternalOutput")

    with tile.TileContext(nc) as tc, ExitStack() as ctx:
        const = ctx.enter_context(tc.tile_pool(name="const", bufs=1))
        pgp = ctx.enter_context(tc.tile_pool(name="pgp", bufs=2))
        mkp = ctx.enter_context(tc.tile_pool(name="mkp", bufs=2))
        ev = ctx.enter_context(tc.tile_pool(name="ev", bufs=2))
        psw = ctx.enter_context(tc.tile_pool(name="psw", bufs=1, space="PSUM"))
        psa = ctx.enter_context(tc.tile_pool(name="psa", bufs=1, space="PSUM"))

        # ---------------- constants / prep ----------------
        eT_sb = const.tile([128, CB, R], bf16)
        nc.sync.dma_start(eT_sb[:], eT_d.ap().rearrange("(co ci) i -> ci co i", ci=128))
        hT_sb = const.tile([D, N], bf16)
        nc.sync.dma_start(hT_sb[:], hT_d.ap())
        hq_sb = const.tile([D, R], bf16)
        nc.sync.dma_start(hq_sb[:], hq_d.ap())
        kwT_sb = const.tile([D, D], bf16)
        nc.sync.dma_start(kwT_sb[:], kwT_d.ap())
        qwT_sb = const.tile([D, D], bf16)
        nc.sync.dma_start(qwT_sb[:], qwT_d.ap())
        vwT_sb = const.tile([D, D], bf16)
        nc.sync.dma_start(vwT_sb[:], vwT_d.ap())
        kb_sb = const.tile([D, 1], f32)
        nc.sync.dma_start(kb_sb[:], kb_d.ap())
        qb_sb = const.tile([D, 1], f32)
        nc.sync.dma_start(qb_sb[:], qb_d.ap())
        vb_sb = const.tile([1, D], bf16)
        nc.sync.dma_start(vb_sb[:], vb_d.ap())
        pb_sb = const.tile([D, JB], f32)
        nc.sync.dma_start(pb_sb[:], pb_d.ap())
        gb_sb = const.tile([D, JB], f32)
        nc.sync.dma_start(gb_sb[:], gb_d.ap())

        onesc_sb = const.tile([128, 1], bf16)
        nc.any.memset(onesc_sb[:], 1.0)
        ones1_sb = const.tile([1, 128], bf16)
        nc.any.memset(ones1_sb[:], 1.0)

        kT_sb = const.tile([D, N], bf16)
        qT_sb = const.tile([D, R], bf16)
        v_sb = const.tile([128, CB, D], bf16)

        # kT[d, j] = Kw @ h.T + Kb  (bias per-partition d via ACT)
        for t in range(N // 512):
            ps = psw.tile([128, 512], f32, tag="s0")
            nc.tensor.matmul(ps[:], kwT_sb[:], hT_sb[:, t * 512:(t + 1) * 512],
                             start=True, stop=True)
            nc.scalar.activation(kT_sb[:, t * 512:(t + 1) * 512], ps[:],
                                 AF.Identity, bias=kb_sb[:])
        # qT[d, i] (pre-scaled by 1/sqrt(D) on host)
        for t, (o, w) in enumerate(((0, I0), (I0, I1))):
            ps = psw.tile([128, 512], f32, tag="s1")
            nc.tensor.matmul(ps[:, :w], qwT_sb[:], hq_sb[:, o:o + w],
                             start=True, stop=True)
            nc.scalar.activation(qT_sb[:, o:o + w], ps[:, :w],
                                 AF.Identity, bias=qb_sb[:])
        # v[j, d] = h @ Vw.T + Vb (bias via K=1 ones matmul)
        for t in range(CB):
            ps = psw.tile([128, 512], f32, tag="g0")
            nc.tensor.matmul(ps[:, :D], hT_sb[:, t * 128:(t + 1) * 128],
                             vwT_sb[:], start=True, stop=False)
            nc.tensor.matmul(ps[:, :D], ones1_sb[:], vb_sb[:],
                             start=False, stop=True)
            nc.any.tensor_copy(v_sb[:, t, :], ps[:, :D])

        # persistent accumulators (4 PSUM banks)
        r0 = psa.tile([1, I0], f32, tag="r0")
        r1 = psa.tile([1, I1], f32, tag="r1")
        o0 = psa.tile([128, I0], f32, tag="o0")
        o1 = psa.tile([128, I1], f32, tag="o1")

        # ---------------- main loop over key blocks ----------------
        for jb in range(JB):
            pgt = pgp.tile([128, CB, 256], bf16)
            nc.sync.dma_start(pgt[:],
                              pg_d.ap()[jb].rearrange("(co ci) j -> ci co j", ci=128))
            mkt = mkp.tile([128, R], bf16)
            nc.sync.dma_start(mkt[:], mk_d.ap()[jb * 128:(jb + 1) * 128, :])

            s0 = psw.tile([128, I0], f32, tag="s0")
            s1 = psw.tile([128, I1], f32, tag="s1")
            g0 = psw.tile([128, I0], f32, tag="g0")
            g1 = psw.tile([128, I1], f32, tag="g1")

            ksl = kT_sb[:, jb * 128:(jb + 1) * 128]
            nc.tensor.matmul(s0[:], ksl, qT_sb[:, 0:I0], start=True, stop=False)
            nc.tensor.matmul(s1[:], ksl, qT_sb[:, I0:R], start=True, stop=False)
            for cb in range(CB):
                lp = pgt[:, cb, 0:128]
                lg = pgt[:, cb, 128:256]
                e0 = eT_sb[:, cb, 0:I0]
                e1 = eT_sb[:, cb, I0:R]
                last = cb == CB - 1
                nc.tensor.matmul(s0[:], lp, e0, start=False, stop=last)
                nc.tensor.matmul(s1[:], lp, e1, start=False, stop=last)
                nc.tensor.matmul(g0[:], lg, e0, start=(cb == 0), stop=last)
                nc.tensor.matmul(g1[:], lg, e1, start=(cb == 0), stop=last)

            expt = ev.tile([128, R], bf16, tag="exp")
            gat = ev.tile([128, R], bf16, tag="gate")
            pbj = pb_sb[:, jb:jb + 1]
            gbj = gb_sb[:, jb:jb + 1]
            nc.scalar.activation(expt[:, 0:I0], s0[:], AF.Exp, bias=pbj)
            nc.scalar.activation(expt[:, I0:R], s1[:], AF.Exp, bias=pbj)
            nc.scalar.activation(gat[:, 0:I0], g0[:], AF.Sigmoid, bias=gbj)
            nc.scalar.activation(gat[:, I0:R], g1[:], AF.Sigmoid, bias=gbj)

            wn = ev.tile([128, R], bf16, tag="wn")
            nc.vector.tensor_tensor(wn[:], expt[:], gat[:], MUL)
            nc.vector.tensor_tensor(wn[:], wn[:], mkt[:], MUL)

            first = jb == 0
            last = jb == JB - 1
            nc.tensor.matmul(r0[:], onesc_sb[:], expt[:, 0:I0],
                             start=first, stop=last, skip_group_check=True)
            nc.tensor.matmul(r1[:], onesc_sb[:], expt[:, I0:R],
                             start=first, stop=last, skip_group_check=True)
            vsl = v_sb[:, jb, :]
            nc.tensor.matmul(o0[:], vsl, wn[:, 0:I0],
                             start=first, stop=last, skip_group_check=True)
            nc.tensor.matmul(o1[:], vsl, wn[:, I0:R],
                             start=first, stop=last, skip_group_check=True)

        # ---------------- finale: evict accumulators (host divides) ----------
        rs = const.tile([1, R], f32)
        nc.vector.tensor_copy(rs[:, 0:I0], r0[:])
        nc.vector.tensor_copy(rs[:, I0:R], r1[:])
        ot = const.tile([128, R], f32)
        nc.vector.tensor_copy(ot[:, 0:I0], o0[:])
        nc.vector.tensor_copy(ot[:, I0:R], o1[:])
        nc.sync.dma_start(rsum_d.ap(), rs[:])
        nc.sync.dma_start(outT_d.ap(), ot[:])

    nc.compile()
    return nc


def _get_nc():
    if "nc" not in _cache:
        _cache["nc"] = _build_nc()
    return _cache["nc"]


def _dropout_mask_T():
    """mask.T, scaled by 1/(1-p): exact replica of the reference dropout."""
    if "maskT" not in _cache:
        import jax
        with jax.default_device(jax.devices("cpu")[0]):
            keep = jax.random.bernoulli(
                jax.random.key(DROP_KEY), 1.0 - DROPOUT_P, (N, N))
            keep = np.asarray(keep)
        _cache["maskT"] = np.ascontiguousarray(
            keep.T.astype(np.float32) / (1.0 - DROPOUT_P)).astype(BF)
    return _cache["maskT"]


def kernel(h, e, Qw, Qb, Kw, Kb, Vw, Vb, Pw, Pb, Gw, Gb):
    from concourse import bass_utils

    nc = _get_nc()
    s = np.float32(1.0 / np.sqrt(D))

    pg = np.empty((JB, N, 256), dtype=BF)
    pg[:, :, :128] = Pw.reshape(JB, 128, N).transpose(0, 2, 1).astype(BF)
    pg[:, :, 128:] = Gw.reshape(JB, 128, N).transpose(0, 2, 1).astype(BF)

    hT = np.ascontiguousarray(h.T).astype(BF)
    maskT = _dropout_mask_T()

    shared = {
        "pg": pg,
        "hT": hT,
        "kwT": np.ascontiguousarray(Kw.T).astype(BF),
        "qwT": np.ascontiguousarray(Qw.T * s).astype(BF),
        "vwT": np.ascontiguousarray(Vw.T).astype(BF),
        "kb": Kb.reshape(D, 1).astype(np.float32),
        "qb": (Qb * s).reshape(D, 1).astype(np.float32),
        "vb": Vb.reshape(1, D).astype(BF),
        "pb": np.ascontiguousarray(Pb.reshape(JB, 128).T).astype(np.float32),
        "gb": np.ascontiguousarray(Gb.reshape(JB, 128).T).astype(np.float32),
    }
    in_maps = []
    for c in range(NCORES):
        rows = slice(c * R, (c + 1) * R)
        in_maps.append({
            **shared,
            "eT": np.ascontiguousarray(e[rows].T).astype(BF),
            "mk": np.ascontiguousarray(maskT[:, rows]),
            "hq": np.ascontiguousarray(hT[:, rows]),
        })

    _cache["in_maps"] = in_maps
    res = bass_utils.run_bass_kernel_spmd(nc, in_maps, core_ids=list(range(NCORES)))
    out = np.empty((N, D), dtype=np.float32)
    for c in range(NCORES):
        r = res.results[c]
        out[c * R:(c + 1) * R] = (r["outT"] / r["rsum"]).T
    return out


# revision 18
# speedup vs baseline: 1.0609x; 1.0609x over previous
"""Trainium2 Bass kernel for gated graph-attention (nn_Att_23502061043766).

Computation (reference, fp32):
    q = h @ Qw.T + Qb; k = h @ Kw.T + Kb; v = h @ Vw.T + Vb
    gate = sigmoid(e @ Gw.T + Gb); pe = e @ Pw.T + Pb
    scores = q @ k.T / sqrt(128) + pe
    attm = softmax(scores, axis=-1); attm = dropout(attm, p=0.2, key=42)
    out = (gate * attm) @ v

Strategy: shard rows of e/scores across the 8 cores (768 rows each). On
each core everything is computed in TRANSPOSED layout sT[j, i] (j = key
index on partitions, i = query rows of this core on the free axis), so:
  - the big [N,N]x[N,N] matmuls stream Pw.T/Gw.T tiles as the stationary
    operand and cached e.T tiles as the moving operand (each weight byte is
    read from HBM exactly once per core),
  - the Pb/Gb biases become per-partition biases folded into the ScalarE
    activation (Exp / Sigmoid) that evicts PSUM,
  - softmax needs no transposes: scores are ~N(0,2) so exp() without the
    row-max subtraction is safe in fp32; the row sum is a ones-vector
    matmul; attention-prob tiles are already [j, i] = exactly the layout
    the final (gate*attm) @ v matmul needs as its moving operand.
All big operands are bf16 (PSUM accumulates fp32).
"""

import sys

for _p in ("/opt/trn_rl_repo",):
    if _p not in sys.path:
        sys.path.append(_p)

import numpy as np
import ml_dtypes

BF = ml_dtypes.bfloat16
N = 6144
D = 128
NCORES = 8
R = N // NCORES          # 768 rows per core
CB = N // 128            # 48 contraction blocks
JB = N // 128            # 48 j blocks
I0, I1 = 512, 256        # i-chunk split of R (PSUM bank is 512 fp32)
DROPOUT_P = 0.2
DROP_KEY = 42

_cache = {}


def _build_nc():
    import concourse.bass as bass  # noqa: F401
    import concourse.tile as tile
    from concourse import bacc, mybir
    from contextlib import ExitStack

    f32 = mybir.dt.float32
    bf16 = mybir.dt.bfloat16
    AF = mybir.ActivationFunctionType
    MUL = mybir.AluOpType.mult

    nc = bacc.Bacc("TRN2", target_bir_lowering=False, debug=False,
                   num_devices=NCORES)

    # DRAM I/O (per-core shapes; per-core data differs only for eT/mk/hq)
    eT_d = nc.dram_tensor("eT", [N, R], bf16, kind="ExternalInput")
    pg_d = nc.dram_tensor("pg", [JB, N, 256], bf16, kind="ExternalInput")
    mk_d = nc.dram_tensor("mk", [N, R], bf16, kind="ExternalInput")
    hT_d = nc.dram_tensor("hT", [D, N], bf16, kind="ExternalInput")
    hq_d = nc.dram_tensor("hq", [D, R], bf16, kind="ExternalInput")
    kwT_d = nc.dram_tensor("kwT", [D, D], bf16, kind="ExternalInput")
    qwT_d = nc.dram_tensor("qwT", [D, D], bf16, kind="ExternalInput")
    vwT_d = nc.dram_tensor("vwT", [D, D], bf16, kind="ExternalInput")
    kb_d = nc.dram_tensor("kb", [D, 1], f32, kind="ExternalInput")
    qb_d = nc.dram_tensor("qb", [D, 1], f32, kind="ExternalInput")
    vb_d = nc.dram_tensor("vb", [1, D], bf16, kind="ExternalInput")
    pb_d = nc.dram_tensor("pb", [D, JB], f32, kind="ExternalInput")
    gb_d = nc.dram_tensor("gb", [D, JB], f32, kind="ExternalInput")
    outT_d = nc.dram_tensor("outT", [D, R], f32, kind="ExternalOutput")
    rsum_d = nc.dram_tensor("rsum", [1, R], f32, kind="ExternalOutput")

    with tile.TileContext(nc) as tc, ExitStack() as ctx:
        const = ctx.enter_context(tc.tile_pool(name="const", bufs=1))
        pgp = ctx.enter_context(tc.tile_pool(name="pgp", bufs=2))
        mkp = ctx.enter_context(tc.tile_pool(name="mkp", bufs=2))
        ev = ctx.enter_context(tc.tile_pool(name="ev", bufs=2))
        psw = ctx.enter_context(tc.tile_pool(name="psw", bufs=1, space="PSUM"))
        psa = ctx.enter_context(tc.tile_pool(name="psa", bufs=1, space="PSUM"))

        # ---------------- constants / prep (small inputs first) ----------------
        hT_sb = const.tile([D, N], bf16)
        nc.sync.dma_start(hT_sb[:], hT_d.ap())
        hq_sb = const.tile([D, R], bf16)
        nc.sync.dma_start(hq_sb[:], hq_d.ap())
        kwT_sb = const.tile([D, D], bf16)
        nc.sync.dma_start(kwT_sb[:], kwT_d.ap())
        qwT_sb = const.tile([D, D], bf16)
        nc.sync.dma_start(qwT_sb[:], qwT_d.ap())
        vwT_sb = const.tile([D, D], bf16)
        nc.sync.dma_start(vwT_sb[:], vwT_d.ap())
        kb_sb = const.tile([D, 1], f32)
        nc.sync.dma_start(kb_sb[:], kb_d.ap())
        qb_sb = const.tile([D, 1], f32)
        nc.sync.dma_start(qb_sb[:], qb_d.ap())
        vb_sb = const.tile([1, D], bf16)
        nc.sync.dma_start(vb_sb[:], vb_d.ap())
        pb_sb = const.tile([D, JB], f32)
        nc.sync.dma_start(pb_sb[:], pb_d.ap())
        gb_sb = const.tile([D, JB], f32)
        nc.sync.dma_start(gb_sb[:], gb_d.ap())

        onesc_sb = const.tile([128, 1], bf16)
        nc.any.memset(onesc_sb[:], 1.0)
        ones1_sb = const.tile([1, 128], bf16)
        nc.any.memset(ones1_sb[:], 1.0)

        # e.T streamed in groups so the first j-block can chase the DMA
        EG = 4
        NEG = CB // EG
        eT_g = []
        for g in range(NEG):
            t = const.tile([128, EG, R], bf16, tag=f"eT{g}")
            nc.sync.dma_start(
                t[:],
                eT_d.ap()[g * EG * 128:(g + 1) * EG * 128, :]
                .rearrange("(co ci) i -> ci co i", ci=128))
            eT_g.append(t)

        kT_sb = const.tile([D, N], bf16)
        qT_sb = const.tile([D, R], bf16)
        v_sb = const.tile([128, CB, D], bf16)

        # kT[d, j] = Kw @ h.T + Kb  (bias per-partition d via ACT)
        for t in range(N // 512):
            ps = psw.tile([128, 512], f32, tag="s0")
            nc.tensor.matmul(ps[:], kwT_sb[:], hT_sb[:, t * 512:(t + 1) * 512],
                             start=True, stop=True)
            nc.scalar.activation(kT_sb[:, t * 512:(t + 1) * 512], ps[:],
                                 AF.Identity, bias=kb_sb[:])
        # qT[d, i] (pre-scaled by 1/sqrt(D) on host)
        for t, (o, w) in enumerate(((0, I0), (I0, I1))):
            ps = psw.tile([128, 512], f32, tag="s1")
            nc.tensor.matmul(ps[:, :w], qwT_sb[:], hq_sb[:, o:o + w],
                             start=True, stop=True)
            nc.scalar.activation(qT_sb[:, o:o + w], ps[:, :w],
                                 AF.Identity, bias=qb_sb[:])
        # v[j, d] = h @ Vw.T + Vb (bias via K=1 ones matmul)
        for t in range(CB):
            ps = psw.tile([128, 512], f32, tag="g0")
            nc.tensor.matmul(ps[:, :D], hT_sb[:, t * 128:(t + 1) * 128],
                             vwT_sb[:], start=True, stop=False)
            nc.tensor.matmul(ps[:, :D], ones1_sb[:], vb_sb[:],
                             start=False, stop=True)
            nc.any.tensor_copy(v_sb[:, t, :], ps[:, :D])

        # persistent accumulators (4 PSUM banks)
        r0 = psa.tile([1, I0], f32, tag="r0")
        r1 = psa.tile([1, I1], f32, tag="r1")
        o0 = psa.tile([128, I0], f32, tag="o0")
        o1 = psa.tile([128, I1], f32, tag="o1")

        # ---------------- main loop over key blocks ----------------
        # Each jb runs as two single-function sweeps: all score matmuls, then
        # all gate matmuls. The score PSUMs complete at mid-iteration, so
        # their Exp eviction overlaps the gate sweep (and vice versa across
        # the jb boundary) — single-buffered PSUM with no eviction stalls.
        # The (gate*attm)@v matmuls of iteration jb are emitted after
        # iteration jb+1's score sweep so the PE never waits on the
        # ACT+DVE chain that produces wn.
        wn_prev = None
        for jb in range(JB):
            pgt = pgp.tile([128, CB, 256], bf16)
            nc.sync.dma_start(pgt[:],
                              pg_d.ap()[jb].rearrange("(co ci) j -> ci co j", ci=128))
            mkt = mkp.tile([128, R], bf16)
            nc.sync.dma_start(mkt[:], mk_d.ap()[jb * 128:(jb + 1) * 128, :])

            s0 = psw.tile([128, I0], f32, tag="s0")
            s1 = psw.tile([128, I1], f32, tag="s1")
            g0 = psw.tile([128, I0], f32, tag="g0")
            g1 = psw.tile([128, I1], f32, tag="g1")

            ksl = kT_sb[:, jb * 128:(jb + 1) * 128]
            nc.tensor.matmul(s0[:], ksl, qT_sb[:, 0:I0], start=True, stop=False)
            nc.tensor.matmul(s1[:], ksl, qT_sb[:, I0:R], start=True, stop=False)
            for cb in range(CB):
                lp = pgt[:, cb, 0:128]
                eg = eT_g[cb // EG]
                last = cb == CB - 1
                nc.tensor.matmul(s0[:], lp, eg[:, cb % EG, 0:I0],
                                 start=False, stop=last)
                nc.tensor.matmul(s1[:], lp, eg[:, cb % EG, I0:R],
                                 start=False, stop=last)

            # previous iteration's output-side matmuls (operands long ready)
            if wn_prev is not None:
                vsl = v_sb[:, jb - 1, :]
                nc.tensor.matmul(o0[:], vsl, wn_prev[:, 0:I0],
                                 start=(jb == 1), stop=False,
                                 skip_group_check=True)
                nc.tensor.matmul(o1[:], vsl, wn_prev[:, I0:R],
                                 start=(jb == 1), stop=False,
                                 skip_group_check=True)

            for cb in range(CB):
                lg = pgt[:, cb, 128:256]
                eg = eT_g[cb // EG]
                last = cb == CB - 1
                nc.tensor.matmul(g0[:], lg, eg[:, cb % EG, 0:I0],
                                 start=(cb == 0), stop=last)
                nc.tensor.matmul(g1[:], lg, eg[:, cb % EG, I0:R],
                                 start=(cb == 0), stop=last)

            expt = ev.tile([128, R], bf16, tag="exp")
            gat = ev.tile([128, R], bf16, tag="gate")
            pbj = pb_sb[:, jb:jb + 1]
            gbj = gb_sb[:, jb:jb + 1]
            nc.scalar.activation(expt[:, 0:I0], s0[:], AF.Exp, bias=pbj)
            nc.scalar.activation(expt[:, I0:R], s1[:], AF.Exp, bias=pbj)
            nc.scalar.activation(gat[:, 0:I0], g0[:], AF.Sigmoid, bias=gbj)
            nc.scalar.activation(gat[:, I0:R], g1[:], AF.Sigmoid, bias=gbj)

            wn = ev.tile([128, R], bf16, tag="wn")
            nc.vector.tensor_tensor(wn[:], expt[:], gat[:], MUL)
            nc.vector.tensor_tensor(wn[:], wn[:], mkt[:], MUL)
            wn_prev = wn

            first = jb == 0
            last = jb == JB - 1
            nc.tensor.matmul(r0[:], onesc_sb[:], expt[:, 0:I0],
                             start=first, stop=last, skip_group_check=True)
            nc.tensor.matmul(r1[:], onesc_sb[:], expt[:, I0:R],
                             start=first, stop=last, skip_group_check=True)

        # final iteration's output matmuls
        vsl = v_sb[:, JB - 1, :]
        nc.tensor.matmul(o0[:], vsl, wn_prev[:, 0:I0],
                         start=False, stop=True, skip_group_check=True)
        nc.tensor.matmul(o1[:], vsl, wn_prev[:, I0:R],
                         start=False, stop=True, skip_group_check=True)

        # ---------------- finale: evict accumulators (host divides) ----------
        rs = const.tile([1, R], f32)
        nc.vector.tensor_copy(rs[:, 0:I0], r0[:])
        nc.vector.tensor_copy(rs[:, I0:R], r1[:])
        ot = const.tile([128, R], f32)
        nc.vector.tensor_copy(ot[:, 0:I0], o0[:])
        nc.vector.tensor_copy(ot[:, I0:R], o1[:])
        nc.sync.dma_start(rsum_d.ap(), rs[:])
        nc.sync.dma_start(outT_d.ap(), ot[:])

    nc.compile()
    return nc


def _get_nc():
    if "nc" not in _cache:
        _cache["nc"] = _build_nc()
    return _cache["nc"]


def _dropout_mask_T():
    """mask.T, scaled by 1/(1-p): exact replica of the reference dropout."""
    if "maskT" not in _cache:
        import jax
        with jax.default_device(jax.devices("cpu")[0]):
            keep = jax.random.bernoulli(
                jax.random.key(DROP_KEY), 1.0 - DROPOUT_P, (N, N))
            keep = np.asarray(keep)
        _cache["maskT"] = np.ascontiguousarray(
            keep.T.astype(np.float32) / (1.0 - DROPOUT_P)).astype(BF)
    return _cache["maskT"]


def kernel(h, e, Qw, Qb, Kw, Kb, Vw, Vb, Pw, Pb, Gw, Gb):
    from concourse import bass_utils

    nc = _get_nc()
    s = np.float32(1.0 / np.sqrt(D))

    pg = np.empty((JB, N, 256), dtype=BF)
    pg[:, :, :128] = Pw.reshape(JB, 128, N).transpose(0, 2, 1).astype(BF)
    pg[:, :, 128:] = Gw.reshape(JB, 128, N).transpose(0, 2, 1).astype(BF)

    hT = np.ascontiguousarray(h.T).astype(BF)
    maskT = _dropout_mask_T()

    shared = {
        "pg": pg,
        "hT": hT,
        "kwT": np.ascontiguousarray(Kw.T).astype(BF),
        "qwT": np.ascontiguousarray(Qw.T * s).astype(BF),
        "vwT": np.ascontiguousarray(Vw.T).astype(BF),
        "kb": Kb.reshape(D, 1).astype(np.float32),
        "qb": (Qb * s).reshape(D, 1).astype(np.float32),
        "vb": Vb.reshape(1, D).astype(BF),
        "pb": np.ascontiguousarray(Pb.reshape(JB, 128).T).astype(np.float32),
        "gb": np.ascontiguousarray(Gb.reshape(JB, 128).T).astype(np.float32),
    }
    in_maps = []
    for c in range(NCORES):
        rows = slice(c * R, (c + 1) * R)
        in_maps.append({
            **shared,
            "eT": np.ascontiguousarray(e[rows].T).astype(BF),
            "mk": np.ascontiguousarray(maskT[:, rows]),
            "hq": np.ascontiguousarray(hT[:, rows]),
        })

    _cache["in_maps"] = in_maps
    res = bass_utils.run_bass_kernel_spmd(nc, in_maps, core_ids=list(range(NCORES)))
    out = np.empty((N, D), dtype=np.float32)
    for c in range(NCORES):
        r = res.results[c]
        out[c * R:(c + 1) * R] = (r["outT"] / r["rsum"]).T
    return out
